# revision 1
# baseline (speedup 1.0000x reference)
"""Trainium2 Bass kernel for nn_DiscreteDecisionEngine.

Math: the reference computes
    q = tanh(geodesic_weights)            # [1, N, 4], N = 256
    h = L(q) (x)  (quaternion Hamilton product per 4-group)
    logits = h_flat @ W.T + b
The Hamilton product is a block-diagonal (4x4 per group) linear map B(q)
applied to x, so logits = x @ (W @ B)^T + b. We fold W' = W @ B on the
host (tiny: [256,1024] weights) and run a pure GEMM on 8 NeuronCores,
data-parallel over the batch.

Device kernel per core (x shard [8192, 1024] f32), DMA-stream-bound:
  for each group of 2 row-tiles (1 MB DMA in, on the SP HWDGE ring):
    per 128-row tile: PE-transpose 128x128 chunks (fp32, 4 per PSUM bank),
    DVE cast-copy -> fp32r (TF32) SBUF, 8 accumulating fp32r matmuls
    psum[128b, 256a] += xT_k.T @ W'T_k, DVE bias-add fused with copyback
    group store [128, 2, 256] via the ACT HWDGE ring
  (software-pipelined one group ahead; last 4 tiles emitted solo to
  shorten the drain)
"""

import os
from contextlib import ExitStack

import numpy as np

import concourse.bass as bass
import concourse.mybir as mybir
import concourse.tile as tile
from concourse import bacc
from concourse.bass import ts
from concourse.bass_utils import run_bass_kernel_spmd
from concourse.masks import make_identity

N_CORES = 8
B_FULL = 65536
B_SHARD = B_FULL // N_CORES  # 8192
D = 1024
A = 256  # num actions
KC = D // 128  # 8 contraction chunks

_F32 = mybir.dt.float32
_F32R = mybir.dt.float32r
_F16 = mybir.dt.float16

# tuning knobs (overridable via env for A/B experiments)
_ACT_COPY_BANK = int(os.environ.get("K_ACT_COPY_BANK", "-1"))
_PIPE = int(os.environ.get("K_PIPE", "1"))
_GROUP = int(os.environ.get("K_GROUP", "2"))  # batch tiles per DMA
_OUT_ON_ACT = bool(int(os.environ.get("K_OUT_ON_ACT", "1")))
_FIRST_SPLIT = int(os.environ.get("K_FIRST_SPLIT", "1024"))  # cols of first sub-load
_TAIL_SPLIT = int(os.environ.get("K_TAIL_SPLIT", "6"))  # trailing tiles emitted solo
_IN_ALT_RING = bool(int(os.environ.get("K_IN_ALT_RING", "0")))
_W_FP16 = bool(int(os.environ.get("K_W_FP16", "1")))  # ship W' as fp16 (exact in TF32)
_TAIL_COLSPLIT = int(os.environ.get("K_TAIL_COLSPLIT", "0"))  # tail groups w/ split loads
_HEAD_SPLIT = int(os.environ.get("K_HEAD_SPLIT", "0"))  # leading tiles emitted solo
_TAIL_ACT = bool(int(os.environ.get("K_TAIL_ACT", "1")))  # ACT copyback in the drain
_DRAIN_FINE = bool(int(os.environ.get("K_DRAIN_FINE", "0")))  # 2-chunk drain copies
_DRAIN_STORE_SP = bool(int(os.environ.get("K_DRAIN_STORE_SP", "1")))  # drain stores on SP ring
_HOLD_STORES = int(os.environ.get("K_HOLD_STORES", "0"))  # early groups' stores deferred to drain
_X16 = bool(int(os.environ.get("K_X16", "0")))  # cast x to fp16, fp16 transpose+matmul
_X16_DVE_MOD = int(os.environ.get("K_X16_DVE_MOD", "2"))  # every Nth group casts on DVE
_BUFS_XIN = int(os.environ.get("K_BUFS_XIN", "5"))
_BUFS_TP = int(os.environ.get("K_BUFS_TP", "4"))
_BUFS_XT = int(os.environ.get("K_BUFS_XT", "4"))
_BUFS_PO = int(os.environ.get("K_BUFS_PO", "3"))
_BUFS_OB = int(os.environ.get("K_BUFS_OB", "4"))


def _build_nc():
    nc = bacc.Bacc(None, target_bir_lowering=False)

    x = nc.dram_tensor("x", [B_SHARD, D], _F32, kind="ExternalInput")
    # w[p, k*A + a] = W'[a, 128*k + p]  (host-prepared, SBUF layout).
    # fp16 halves the transfer; its 11-bit significand matches TF32, so the
    # device-side upconvert to f32r is exact for these magnitudes.
    w = nc.dram_tensor("w", [128, KC * A], _F16 if _W_FP16 else _F32R,
                       kind="ExternalInput")
    # bias broadcast to all 128 partitions on host
    bias = nc.dram_tensor("bias", [128, A], _F32, kind="ExternalInput")
    out = nc.dram_tensor("out", [B_SHARD, A], _F32, kind="ExternalOutput")

    with ExitStack() as ctx:
        tc = ctx.enter_context(tile.TileContext(nc))
        const = ctx.enter_context(tc.tile_pool(name="const", bufs=1))

        xin = ctx.enter_context(tc.tile_pool(name="xin", bufs=_BUFS_XIN))
        tp = ctx.enter_context(tc.tile_pool(name="tp", bufs=_BUFS_TP, space="PSUM"))
        xt = ctx.enter_context(tc.tile_pool(name="xt", bufs=_BUFS_XT))
        po = ctx.enter_context(tc.tile_pool(name="po", bufs=_BUFS_PO, space="PSUM"))
        ob = ctx.enter_context(tc.tile_pool(name="ob", bufs=_BUFS_OB))
        obh = (
            ctx.enter_context(tc.tile_pool(name="obh", bufs=_HOLD_STORES))
            if _HOLD_STORES > 0
            else None
        )

        n_tiles = B_SHARD // 128
        G = _GROUP
        # schedule of (first_tile, group_size); head/tail split into
        # single-tile groups to start the PE earlier / shorten the drain
        head = min(_HEAD_SPLIT, n_tiles)
        tail = min(_TAIL_SPLIT, n_tiles - head)
        main_tiles = n_tiles - head - tail
        assert main_tiles % G == 0
        sched = [(j, 1) for j in range(head)]
        sched += [(head + i * G, G) for i in range(main_tiles // G)]
        sched += [(head + main_tiles + j, 1) for j in range(tail)]
        n_groups = len(sched)
        staged = {}

        # first x load is issued before the (1MB) weight load so the PE's
        # transposes start as early as possible; ident is device-generated
        ident = const.tile([128, 128], _F32)
        make_identity(nc, ident)
        g0 = sched[0][1]
        xg0 = xin.tile([128, g0, D], _F32, tag=f"xg{g0}")
        src0 = x[bass.ds(0, g0 * 128), :]
        if g0 > 1:
            src0 = src0.rearrange("(t p) d -> p t d", p=128)
        else:
            src0 = src0.rearrange("p (t d) -> p t d", t=1)
        nc.sync.dma_start(xg0[:, 0, ts(0, _FIRST_SPLIT)], src0[:, 0, ts(0, _FIRST_SPLIT)])
        if _FIRST_SPLIT < D:
            nc.sync.dma_start(
                xg0[:, 0, _FIRST_SPLIT:], src0[:, 0, _FIRST_SPLIT:]
            )
        for t in range(1, g0):
            nc.sync.dma_start(xg0[:, t, :], src0[:, t, :])

        # weights/bias ride the ACT HWDGE ring (idle at startup) so they
        # don't delay the x stream on the SP ring
        if _X16:
            # matmul consumes fp16 weights directly; drain tiles stay on the
            # f32r path (no cast stage in their latency chain), so keep both
            w16 = const.tile([128, KC, A], _F16)
            nc.scalar.dma_start(w16[:], w.rearrange("p (k a) -> p k a", k=KC))
            w_sb = const.tile([128, KC, A], _F32R)
            nc.vector.tensor_copy(out=w_sb[:], in_=w16[:])
            w_mm = w_sb
            ident16 = const.tile([128, 128], _F16)
            make_identity(nc, ident16)
        elif _W_FP16:
            w_sb = const.tile([128, KC, A], _F32R)
            w16 = const.tile([128, KC, A], _F16)
            nc.scalar.dma_start(w16[:], w.rearrange("p (k a) -> p k a", k=KC))
            nc.vector.tensor_copy(out=w_sb[:], in_=w16[:])
            w_mm = w_sb
        else:
            w_sb = const.tile([128, KC, A], _F32R)
            nc.scalar.dma_start(w_sb[:], w.rearrange("p (k a) -> p k a", k=KC))
            w_mm = w_sb
        bias_sb = const.tile([128, A], _F32)
        nc.scalar.dma_start(bias_sb[:], bias[:])

        def stage_load_transpose(gi):
            row0, g = sched[gi]
            if gi == 0:
                xg = xg0
            else:
                xg = xin.tile([128, g, D], _F32, tag=f"xg{g}")
                src = x[ts(row0, 128) if g == 1 else bass.ds(row0 * 128, g * 128), :]
                if g > 1:
                    src = src.rearrange("(t p) d -> p t d", p=128)
                else:
                    src = src.rearrange("p (t d) -> p t d", t=1)
                if _IN_ALT_RING and gi % 2 == 1:
                    nc.scalar.dma_start(xg[:], src)
                elif g == 1 and gi >= n_groups - _TAIL_COLSPLIT:
                    # split the last loads by column halves so the drain's
                    # transposes start before the full tile lands
                    nc.sync.dma_start(xg[:, :, : D // 2], src[:, :, : D // 2])
                    nc.sync.dma_start(xg[:, :, D // 2 :], src[:, :, D // 2 :])
                else:
                    nc.sync.dma_start(xg[:], src)
            xts = []
            in_drain = _TAIL_ACT and row0 >= n_tiles - _TAIL_SPLIT
            use16 = _X16 and not in_drain
            if use16:
                # cast the group to fp16 (11-bit significand, same as TF32's)
                # on ACT/DVE before the PE transposes; halves PE transpose and
                # DVE copyback time
                xg16 = xin.tile([128, g, D], _F16, tag=f"x16{g}")
                cast_eng = (
                    nc.vector.tensor_copy
                    if (_X16_DVE_MOD > 0 and gi % _X16_DVE_MOD == 0)
                    else nc.scalar.copy
                )
                for t in range(g):
                    cast_eng(out=xg16[:, t, :], in_=xg[:, t, :])
                xg = xg16
            t_ident = ident16 if use16 else ident
            t_dt = _F16 if use16 else _F32
            xt_dt = _F16 if use16 else _F32R
            if in_drain and _DRAIN_FINE:
                for t in range(g):
                    xt_tile = xt.tile([128, KC, 128], xt_dt, tag="xt")
                    for h in range(KC // 2):
                        pt = tp.tile([128, 2, 128], t_dt, tag="pt")
                        for j in range(2):
                            k = 2 * h + j
                            nc.tensor.transpose(
                                pt[:, j, :], xg[:, t, ts(k, 128)], t_ident[:]
                            )
                        if h % 2 == 1:
                            nc.scalar.copy(out=xt_tile[:, ts(h, 2), :], in_=pt[:])
                        else:
                            nc.vector.tensor_copy(
                                out=xt_tile[:, ts(h, 2), :], in_=pt[:]
                            )
                    xts.append(xt_tile)
                staged[gi] = (xts, use16)
                return
            for t in range(g):
                xt_tile = xt.tile([128, KC, 128], xt_dt, tag="xt")
                for g in range(KC // 4):
                    # 4 transposed chunks per PSUM bank -> single wide copyback
                    pt = tp.tile([128, 4, 128], t_dt, tag="pt")
                    for j in range(4):
                        k = 4 * g + j
                        nc.tensor.transpose(
                            pt[:, j, :], xg[:, t, ts(k, 128)], t_ident[:]
                        )
                    # cast-copy f32 -> f32r (TF32 rounding) for the PE;
                    # optionally alternate banks between DVE and ACT
                    in_drain = _TAIL_ACT and row0 >= n_tiles - _TAIL_SPLIT
                    if (_ACT_COPY_BANK >= 0 and g % 2 == _ACT_COPY_BANK) or (
                        in_drain and g % 2 == 1
                    ):
                        nc.scalar.copy(out=xt_tile[:, ts(g, 4), :], in_=pt[:])
                    else:
                        nc.vector.tensor_copy(out=xt_tile[:, ts(g, 4), :], in_=pt[:])
                xts.append(xt_tile)
            staged[gi] = (xts, use16)

        held_stores = []

        def stage_matmul_store(gi):
            row0, g = sched[gi]
            xts, use16 = staged.pop(gi)
            hold = gi < _HOLD_STORES
            if hold:
                og = obh.tile([128, g, A], _F32, tag=f"obh{g}")
            else:
                og = ob.tile([128, g, A], _F32, tag=f"ob{g}")
            for t in range(g):
                p_out = po.tile([128, A], _F32)
                for k in range(KC):
                    nc.tensor.matmul(
                        p_out[:],
                        lhsT=xts[t][:, k, :],
                        rhs=(w16 if use16 else w_mm)[:, k, :],
                        start=(k == 0),
                        stop=(k == KC - 1),
                    )
                nc.vector.tensor_add(og[:, t, :], p_out[:], bias_sb[:])
            dst = out[bass.ds(row0 * 128, g * 128), :]
            if g > 1:
                dst = dst.rearrange("(t p) a -> p t a", p=128)
            else:
                dst = dst.rearrange("p (t a) -> p t a", t=1)
            if hold:
                # store deferred: flushed right before the drain groups so the
                # in-stream finishes earlier and these fill the drain window
                held_stores.append((dst, og))
                return
            drain_store_sp = _DRAIN_STORE_SP and row0 >= n_tiles - _TAIL_SPLIT
            if _OUT_ON_ACT and not drain_store_sp:
                nc.scalar.dma_start(dst, og[:])
            else:
                nc.sync.dma_start(dst, og[:])

        # optional software pipeline: emit transposes of group i+PIPE before
        # matmuls of group i
        first_drain = n_groups - tail
        for i in range(n_groups + _PIPE):
            if i == first_drain and held_stores:
                for dst_h, og_h in held_stores:
                    nc.scalar.dma_start(dst_h, og_h[:])
                held_stores.clear()
            if i < n_groups:
                stage_load_transpose(i)
            if i >= _PIPE:
                stage_matmul_store(i - _PIPE)

    nc.finalize()  # runs Bacc.compile(): wait-splitting etc.
    return nc


_NC_CACHE = None
LAST_RESULTS = None


def _get_nc():
    global _NC_CACHE
    if _NC_CACHE is None:
        _NC_CACHE = _build_nc()
    return _NC_CACHE


def _fold_weights(geodesic_weights: np.ndarray, W: np.ndarray) -> np.ndarray:
    """W' = W @ blockdiag(L(tanh(g))^T per 4-group), in float64."""
    q = np.tanh(geodesic_weights.astype(np.float64))[0]  # [N, 4]
    w_, i_, j_, k_ = q[:, 0], q[:, 1], q[:, 2], q[:, 3]
    n = q.shape[0]
    M = np.empty((n, 4, 4), dtype=np.float64)  # y_r = sum_s M[n, r, s] x_s
    M[:, 0] = np.stack([w_, -i_, -j_, -k_], axis=-1)
    M[:, 1] = np.stack([i_, w_, -k_, j_], axis=-1)
    M[:, 2] = np.stack([j_, k_, w_, -i_], axis=-1)
    M[:, 3] = np.stack([k_, -j_, i_, w_], axis=-1)
    W4 = W.astype(np.float64).reshape(A, n, 4)  # [a, n, r]
    Wp = np.einsum("anr,nrs->ans", W4, M).reshape(A, D)
    return Wp.astype(np.float32)  # [a, d]


def kernel(x, geodesic_weights, W, b, **_unused):
    x = np.ascontiguousarray(np.asarray(x, dtype=np.float32))
    Wp = _fold_weights(np.asarray(geodesic_weights), np.asarray(W))
    # device layout: w_dev[p, k*A + a] = Wp[a, 128k + p]
    w_dev = np.ascontiguousarray(
        Wp.T.reshape(KC, 128, A).transpose(1, 0, 2).reshape(128, KC * A)
    )
    if _W_FP16:
        w_dev = w_dev.astype(np.float16)
    bias_dev = np.ascontiguousarray(
        np.broadcast_to(np.asarray(b, dtype=np.float32)[None, :], (128, A))
    )

    nc = _get_nc()
    shards = np.split(x, N_CORES, axis=0)
    in_maps = [{"x": s, "w": w_dev, "bias": bias_dev} for s in shards]
    res = run_bass_kernel_spmd(
        nc,
        in_maps,
        core_ids=list(range(N_CORES)),
        trace=bool(int(os.environ.get("KERNEL_TRACE", "0"))),
    )
    global LAST_RESULTS
    LAST_RESULTS = res
    out = np.concatenate([r["out"] for r in res.results], axis=0)
    return out



# revision 2
# speedup vs baseline: 1.0227x; 1.0227x over previous
"""Trainium2 Bass kernel for nn_DiscreteDecisionEngine, schedule v7 (fp16 compute + pinned SP endgame).

Math: logits = x @ (W @ B(q))^T + b with B(q) the block-diagonal Hamilton
map; W' = W @ B folded on host, so the device runs a pure GEMM,
data-parallel over batch on 8 cores (x shard [8192, 1024] f32 per core).

Schedule (DMA device is the serialized bottleneck at 360 B/ns):
  SP queue   : x load groups (1 MB), then batch-A store (first H tiles,
               one DMA) emitted right after the last load so it fills the
               DMA pipe while the last tile's compute drains.
  Pool queue : interleaved 2-tile stores for middle tiles, then batch-B
               store (last TB tiles, one DMA) — its SWDGE gen runs after
               the final bias-add and lands right as batch A finishes.
  ACT queue  : w/bias loads, then one PSUM->SBUF transpose-copyback per
               tile; DVE queue: the other copyback + bias-add.
  PE         : 128x128 fp32 transposes (4 per PSUM bank) + 8 accumulating
               f32r matmuls per tile.
Every queue carries a single sem-ordered stream, so the DMA engine always
has a parked transfer: mid-stream it is 100% occupied, and the endgame is
covered by the two batched stores.
"""

import os
from contextlib import ExitStack

import numpy as np

import concourse.bass as bass
import concourse.mybir as mybir
import concourse.tile as tile
from concourse import bacc
from concourse.bass import ts
from concourse.bass_utils import run_bass_kernel_spmd
from concourse.masks import make_identity

N_CORES = 8
B_FULL = 65536
B_SHARD = B_FULL // N_CORES  # 8192
D = 1024
A = 256  # num actions
KC = D // 128  # 8 contraction chunks

_F32 = mybir.dt.float32
_F32R = mybir.dt.float32r
_F16 = mybir.dt.float16

# schedule knobs
_HOLD = int(os.environ.get("K7_HOLD", "16"))      # tiles in batch-A (held) store
_TAILB = int(os.environ.get("K7_TAILB", "6"))     # tiles in batch-B (final) store
_GROUP = 2                                         # batch tiles per load DMA
_BUFS_XIN = int(os.environ.get("K7_BUFS_XIN", "8"))
_BUFS_X16 = int(os.environ.get("K7_BUFS_X16", "6"))
_DEFER = int(os.environ.get("K7_DEFER", "2"))
_BUFS_TP = int(os.environ.get("K7_BUFS_TP", "4"))
_BUFS_XT = int(os.environ.get("K7_BUFS_XT", "6"))
_BUFS_PO = int(os.environ.get("K7_BUFS_PO", "4"))
_BUFS_OB = int(os.environ.get("K7_BUFS_OB", "8"))
_PIPE = int(os.environ.get("K7_PIPE", "1"))
_SPLIT_LAST = int(os.environ.get("K7_SPLIT_LAST", "1"))  # col-split last tile load


def _build_nc():
    nc = bacc.Bacc(None, target_bir_lowering=False)

    x = nc.dram_tensor("x", [B_SHARD, D], _F32, kind="ExternalInput")
    # w[p, k*A + a] = W'[a, 128*k + p]; fp16 halves the transfer and its
    # 11-bit significand matches TF32, so the f32r upconvert is exact.
    w = nc.dram_tensor("w", [128, KC * A], _F16, kind="ExternalInput")
    bias = nc.dram_tensor("bias", [128, A], _F32, kind="ExternalInput")
    out = nc.dram_tensor("out", [B_SHARD, A], _F32, kind="ExternalOutput")

    n_tiles = B_SHARD // 128  # 64
    H = _HOLD
    TB = _TAILB
    first_b = n_tiles - TB  # first tile of batch B

    with ExitStack() as ctx:
        tc = ctx.enter_context(tile.TileContext(nc))
        const = ctx.enter_context(tc.tile_pool(name="const", bufs=1))
        xin = ctx.enter_context(tc.tile_pool(name="xin", bufs=_BUFS_XIN))
        x16p = ctx.enter_context(tc.tile_pool(name="x16p", bufs=_BUFS_X16))
        tp = ctx.enter_context(tc.tile_pool(name="tp", bufs=_BUFS_TP, space="PSUM"))
        xt = ctx.enter_context(tc.tile_pool(name="xt", bufs=_BUFS_XT))
        po = ctx.enter_context(tc.tile_pool(name="po", bufs=_BUFS_PO, space="PSUM"))
        ob = ctx.enter_context(tc.tile_pool(name="ob", bufs=_BUFS_OB))
        obh = ctx.enter_context(tc.tile_pool(name="obh", bufs=1))
        obb = ctx.enter_context(tc.tile_pool(name="obb", bufs=1))

        # persistent output staging for the two batched stores
        obh_t = obh.tile([128, H, A], _F32)
        obb_t = obb.tile([128, TB, A], _F32)

        ident16 = const.tile([128, 128], _F16)
        make_identity(nc, ident16)

        # first x load goes out before anything else on SP
        n_groups = n_tiles // _GROUP
        xgs = {}

        def emit_load(gi):
            g = _GROUP
            row0 = gi * g
            xg = xin.tile([128, g, D], _F32, tag="xg")
            src = x[bass.ds(row0 * 128, g * 128), :].rearrange("(t p) d -> p t d", p=128)
            last = gi == n_groups - 1
            if last and _SPLIT_LAST:
                nc.sync.dma_start(xg[:, 0, :], src[:, 0, :])
                nc.sync.dma_start(xg[:, 1, : D // 2], src[:, 1, : D // 2])
                nc.sync.dma_start(xg[:, 1, D // 2 :], src[:, 1, D // 2 :])
            else:
                nc.sync.dma_start(xg[:], src)
            xgs[gi] = xg

        emit_load(0)

        # weights/bias ride the ACT HWDGE ring; parked before L1 so they run
        # right after L0 without delaying the SP stream's issue
        w16 = const.tile([128, KC, A], _F16)
        nc.scalar.dma_start(w16[:], w.rearrange("p (k a) -> p k a", k=KC))
        bias_sb = const.tile([128, A], _F32)
        nc.scalar.dma_start(bias_sb[:], bias[:])

        staged = {}
        deferred = []  # (row0, og) group stores pinned to SP after batch A

        def emit_transpose(gi):
            xg = xgs[gi]
            xts = []
            # cast the group to fp16 on ACT (idle engine); fp16 transposes
            # run at 1 PE cycle/row vs fp32's 2, keeping PE under the pure
            # load pace so held-store load rushes get absorbed
            xg16 = x16p.tile([128, _GROUP, D], _F16, tag="x16")
            for t in range(_GROUP):
                nc.scalar.copy(out=xg16[:, t, :], in_=xg[:, t, :])
            for t in range(_GROUP):
                # all 8 transposed chunks fill one 2KB PSUM bank exactly,
                # evicted with a single wide DVE copy
                xt_tile = xt.tile([128, KC, 128], _F16, tag="xt")
                pt = tp.tile([128, KC, 128], _F16, tag="pt")
                for k in range(KC):
                    nc.tensor.transpose(pt[:, k, :], xg16[:, t, ts(k, 128)], ident16[:])
                nc.vector.tensor_copy(out=xt_tile[:], in_=pt[:])
                xts.append(xt_tile)
            staged[gi] = xts

        def emit_matmul(gi):
            xts = staged.pop(gi)
            row0 = gi * _GROUP
            og = None
            for t in range(_GROUP):
                tile_id = row0 + t
                p_out = po.tile([128, A], _F32)
                for k in range(KC):
                    nc.tensor.matmul(
                        p_out[:],
                        lhsT=xts[t][:, k, :],
                        rhs=w16[:, k, :],
                        start=(k == 0),
                        stop=(k == KC - 1),
                    )
                if tile_id < H:
                    dst_sb = obh_t[:, tile_id, :]
                elif tile_id >= first_b:
                    dst_sb = obb_t[:, tile_id - first_b, :]
                else:
                    if og is None:
                        og = ob.tile([128, _GROUP, A], _F32, tag="ob")
                    dst_sb = og[:, t, :]
                nc.vector.tensor_add(dst_sb, p_out[:], bias_sb[:])
            # middle tiles: interleave on the Pool ring, except the last
            # _DEFER groups which are pinned to SP after batch A
            if og is not None and row0 >= H and row0 + _GROUP <= first_b:
                if row0 + _DEFER * _GROUP >= first_b:
                    deferred.append((row0, og))
                    return
                dst = out[bass.ds(row0 * 128, _GROUP * 128), :].rearrange(
                    "(t p) a -> p t a", p=128
                )
                nc.gpsimd.dma_start(dst, og[:])

        # software pipeline: loads run ahead; transposes of group i+PIPE
        # emitted before matmuls of group i
        for i in range(n_groups + _PIPE + 1):
            if i + 1 < n_groups:
                emit_load(i + 1)
            if i == n_groups - 1:
                # batch-A store emitted on SP right after the last load:
                # FIFO device arbitration runs it after the final x DMA
                dstA = out[bass.ds(0, H * 128), :].rearrange("(t p) a -> p t a", p=128)
                nc.sync.dma_start(dstA, obh_t[:])
            if i < n_groups:
                emit_transpose(i)
            if i - _PIPE >= 0 and i - _PIPE < n_groups:
                emit_matmul(i - _PIPE)

        # pinned SP endgame after batch A: deferred mid groups in order,
        # then batch B (waits the final bias-add) as the very last transfer
        for row0, og in deferred:
            dst = out[bass.ds(row0 * 128, _GROUP * 128), :].rearrange(
                "(t p) a -> p t a", p=128
            )
            nc.sync.dma_start(dst, og[:])
        dstB = out[bass.ds(first_b * 128, TB * 128), :].rearrange(
            "(t p) a -> p t a", p=128
        )
        nc.sync.dma_start(dstB, obb_t[:])

    nc.finalize()
    return nc


_NC_CACHE = None
LAST_RESULTS = None


def _get_nc():
    global _NC_CACHE
    if _NC_CACHE is None:
        _NC_CACHE = _build_nc()
    return _NC_CACHE


def _fold_weights(geodesic_weights: np.ndarray, W: np.ndarray) -> np.ndarray:
    """W' = W @ blockdiag(L(tanh(g))^T per 4-group), in float64."""
    q = np.tanh(geodesic_weights.astype(np.float64))[0]  # [N, 4]
    w_, i_, j_, k_ = q[:, 0], q[:, 1], q[:, 2], q[:, 3]
    n = q.shape[0]
    M = np.empty((n, 4, 4), dtype=np.float64)  # y_r = sum_s M[n, r, s] x_s
    M[:, 0] = np.stack([w_, -i_, -j_, -k_], axis=-1)
    M[:, 1] = np.stack([i_, w_, -k_, j_], axis=-1)
    M[:, 2] = np.stack([j_, k_, w_, -i_], axis=-1)
    M[:, 3] = np.stack([k_, -j_, i_, w_], axis=-1)
    W4 = W.astype(np.float64).reshape(A, n, 4)  # [a, n, r]
    Wp = np.einsum("anr,nrs->ans", W4, M).reshape(A, D)
    return Wp.astype(np.float32)  # [a, d]


def kernel(x, geodesic_weights, W, b, **_unused):
    x = np.ascontiguousarray(np.asarray(x, dtype=np.float32))
    Wp = _fold_weights(np.asarray(geodesic_weights), np.asarray(W))
    # device layout: w_dev[p, k*A + a] = Wp[a, 128k + p]
    w_dev = np.ascontiguousarray(
        Wp.T.reshape(KC, 128, A).transpose(1, 0, 2).reshape(128, KC * A)
    ).astype(np.float16)
    bias_dev = np.ascontiguousarray(
        np.broadcast_to(np.asarray(b, dtype=np.float32)[None, :], (128, A))
    )

    nc = _get_nc()
    shards = np.split(x, N_CORES, axis=0)
    in_maps = [{"x": s, "w": w_dev, "bias": bias_dev} for s in shards]
    res = run_bass_kernel_spmd(
        nc,
        in_maps,
        core_ids=list(range(N_CORES)),
        trace=bool(int(os.environ.get("KERNEL_TRACE", "0"))),
    )
    global LAST_RESULTS
    LAST_RESULTS = res
    out = np.concatenate([r["out"] for r in res.results], axis=0)
    return out


# revision 7
# speedup vs baseline: 1.0257x; 1.0029x over previous
"""Trainium2 Bass kernel for nn_DiscreteDecisionEngine, schedule v7 (fp16 compute + pinned SP endgame).

Math: logits = x @ (W @ B(q))^T + b with B(q) the block-diagonal Hamilton
map; W' = W @ B folded on host, so the device runs a pure GEMM,
data-parallel over batch on 8 cores (x shard [8192, 1024] f32 per core).

Schedule (DMA device is the serialized bottleneck at 360 B/ns):
  SP queue   : x load groups (1 MB), then batch-A store (first H tiles,
               one DMA) emitted right after the last load so it fills the
               DMA pipe while the last tile's compute drains.
  Pool queue : interleaved 2-tile stores for middle tiles, then batch-B
               store (last TB tiles, one DMA) — its SWDGE gen runs after
               the final bias-add and lands right as batch A finishes.
  ACT queue  : w/bias loads, then one PSUM->SBUF transpose-copyback per
               tile; DVE queue: the other copyback + bias-add.
  PE         : 128x128 fp32 transposes (4 per PSUM bank) + 8 accumulating
               f32r matmuls per tile.
Every queue carries a single sem-ordered stream, so the DMA engine always
has a parked transfer: mid-stream it is 100% occupied, and the endgame is
covered by the two batched stores.
"""

import os
from contextlib import ExitStack

import numpy as np

import concourse.bass as bass
import concourse.mybir as mybir
import concourse.tile as tile
from concourse import bacc
from concourse.bass import ts
from concourse.bass_utils import run_bass_kernel_spmd
from concourse.masks import make_identity

N_CORES = 8
B_FULL = 65536
B_SHARD = B_FULL // N_CORES  # 8192
D = 1024
A = 256  # num actions
KC = D // 128  # 8 contraction chunks

_F32 = mybir.dt.float32
_F32R = mybir.dt.float32r
_F16 = mybir.dt.float16

# schedule knobs
_HOLD = int(os.environ.get("K7_HOLD", "16"))      # tiles in batch-A (held) store
_TAILB = int(os.environ.get("K7_TAILB", "6"))     # tiles in batch-B (final) store
_GROUP = 2                                         # batch tiles per load DMA
_BUFS_XIN = int(os.environ.get("K7_BUFS_XIN", "8"))
_BUFS_X16 = int(os.environ.get("K7_BUFS_X16", "6"))
_DEFER = int(os.environ.get("K7_DEFER", "2"))
_BUFS_TP = int(os.environ.get("K7_BUFS_TP", "4"))
_BUFS_XT = int(os.environ.get("K7_BUFS_XT", "6"))
_BUFS_PO = int(os.environ.get("K7_BUFS_PO", "4"))
_BUFS_OB = int(os.environ.get("K7_BUFS_OB", "8"))
_PIPE = int(os.environ.get("K7_PIPE", "1"))
_SPLIT_LAST = int(os.environ.get("K7_SPLIT_LAST", "1"))  # col-split last tile load


def _build_nc():
    nc = bacc.Bacc(None, target_bir_lowering=False)

    x = nc.dram_tensor("x", [B_SHARD, D], _F32, kind="ExternalInput")
    # w[p, k*A + a] = W'[a, 128*k + p]; fp16 halves the transfer and its
    # 11-bit significand matches TF32, so the fp16 matmul is TF32-exact.
    w = nc.dram_tensor("w", [128, KC * A], _F16, kind="ExternalInput")
    # bias ships as a single partition line (1 KB, ~7 ns DMA hold) and is
    # broadcast across partitions on device via a ones-vector PE matmul
    bias = nc.dram_tensor("bias", [1, A], _F32, kind="ExternalInput")
    out = nc.dram_tensor("out", [B_SHARD, A], _F32, kind="ExternalOutput")

    n_tiles = B_SHARD // 128  # 64
    H = _HOLD
    TB = _TAILB
    first_b = n_tiles - TB  # first tile of batch B

    with ExitStack() as ctx:
        tc = ctx.enter_context(tile.TileContext(nc))
        const = ctx.enter_context(tc.tile_pool(name="const", bufs=1))
        xin = ctx.enter_context(tc.tile_pool(name="xin", bufs=_BUFS_XIN))
        x16p = ctx.enter_context(tc.tile_pool(name="x16p", bufs=_BUFS_X16))
        tp = ctx.enter_context(tc.tile_pool(name="tp", bufs=_BUFS_TP, space="PSUM"))
        xt = ctx.enter_context(tc.tile_pool(name="xt", bufs=_BUFS_XT))
        po = ctx.enter_context(tc.tile_pool(name="po", bufs=_BUFS_PO, space="PSUM"))
        ob = ctx.enter_context(tc.tile_pool(name="ob", bufs=_BUFS_OB))
        obh = ctx.enter_context(tc.tile_pool(name="obh", bufs=1))
        obb = ctx.enter_context(tc.tile_pool(name="obb", bufs=1))

        # persistent output staging for the two batched stores
        obh_t = obh.tile([128, H, A], _F32)
        obb_t = obb.tile([128, TB, A], _F32)

        ident16 = const.tile([128, 128], _F16)
        make_identity(nc, ident16)

        # first x load goes out before anything else on SP
        n_groups = n_tiles // _GROUP
        xgs = {}

        def emit_load(gi):
            g = _GROUP
            row0 = gi * g
            xg = xin.tile([128, g, D], _F32, tag="xg")
            src = x[bass.ds(row0 * 128, g * 128), :].rearrange("(t p) d -> p t d", p=128)
            last = gi == n_groups - 1
            if last and _SPLIT_LAST:
                nc.sync.dma_start(xg[:, 0, :], src[:, 0, :])
                nc.sync.dma_start(xg[:, 1, : D // 2], src[:, 1, : D // 2])
                nc.sync.dma_start(xg[:, 1, D // 2 :], src[:, 1, D // 2 :])
            else:
                nc.sync.dma_start(xg[:], src)
            xgs[gi] = xg

        emit_load(0)

        # weights/bias ride the ACT HWDGE ring; parked before L1 so they run
        # right after L0 without delaying the SP stream's issue
        w16 = const.tile([128, KC, A], _F16)
        nc.scalar.dma_start(w16[:], w.rearrange("p (k a) -> p k a", k=KC))
        bias_row = const.tile([1, A], _F32)
        nc.scalar.dma_start(bias_row[:], bias[:])
        ones_row = const.tile([1, 128], _F32)
        nc.vector.memset(ones_row[:], 1.0)
        # transient slot in the p_out ring; freed for reuse by the copy below
        bias_ps = po.tile([128, A], _F32, tag="p_out")
        nc.tensor.matmul(bias_ps[:], lhsT=ones_row[:], rhs=bias_row[:],
                         start=True, stop=True)
        bias_sb = const.tile([128, A], _F32)
        nc.vector.tensor_copy(out=bias_sb[:], in_=bias_ps[:])

        staged = {}
        deferred = []  # (row0, og) group stores pinned to SP after batch A

        def emit_transpose(gi):
            xg = xgs[gi]
            xts = []
            # cast the group to fp16 on ACT (idle engine); fp16 transposes
            # run at 1 PE cycle/row vs fp32's 2, keeping PE under the pure
            # load pace so held-store load rushes get absorbed
            xg16 = x16p.tile([128, _GROUP, D], _F16, tag="x16")
            for t in range(_GROUP):
                nc.scalar.copy(out=xg16[:, t, :], in_=xg[:, t, :])
            for t in range(_GROUP):
                # all 8 transposed chunks fill one 2KB PSUM bank exactly,
                # evicted with a single wide DVE copy
                xt_tile = xt.tile([128, KC, 128], _F16, tag="xt")
                pt = tp.tile([128, KC, 128], _F16, tag="pt")
                for k in range(KC):
                    nc.tensor.transpose(pt[:, k, :], xg16[:, t, ts(k, 128)], ident16[:])
                nc.vector.tensor_copy(out=xt_tile[:], in_=pt[:])
                xts.append(xt_tile)
            staged[gi] = xts

        def emit_matmul(gi):
            xts = staged.pop(gi)
            row0 = gi * _GROUP
            og = None
            for t in range(_GROUP):
                tile_id = row0 + t
                p_out = po.tile([128, A], _F32)
                for k in range(KC):
                    nc.tensor.matmul(
                        p_out[:],
                        lhsT=xts[t][:, k, :],
                        rhs=w16[:, k, :],
                        start=(k == 0),
                        stop=(k == KC - 1),
                    )
                if tile_id < H:
                    dst_sb = obh_t[:, tile_id, :]
                elif tile_id >= first_b:
                    dst_sb = obb_t[:, tile_id - first_b, :]
                else:
                    if og is None:
                        og = ob.tile([128, _GROUP, A], _F32, tag="ob")
                    dst_sb = og[:, t, :]
                nc.vector.tensor_add(dst_sb, p_out[:], bias_sb[:])
            # middle tiles: interleave on the Pool ring, except the last
            # _DEFER groups which are pinned to SP after batch A
            if og is not None and row0 >= H and row0 + _GROUP <= first_b:
                if row0 + _DEFER * _GROUP >= first_b:
                    deferred.append((row0, og))
                    return
                dst = out[bass.ds(row0 * 128, _GROUP * 128), :].rearrange(
                    "(t p) a -> p t a", p=128
                )
                nc.gpsimd.dma_start(dst, og[:])

        # software pipeline: loads run ahead; transposes of group i+PIPE
        # emitted before matmuls of group i
        for i in range(n_groups + _PIPE + 1):
            if i + 1 < n_groups:
                emit_load(i + 1)
            if i == n_groups - 1:
                # batch-A store emitted on SP right after the last load:
                # FIFO device arbitration runs it after the final x DMA
                dstA = out[bass.ds(0, H * 128), :].rearrange("(t p) a -> p t a", p=128)
                nc.sync.dma_start(dstA, obh_t[:])
            if i < n_groups:
                emit_transpose(i)
            if i - _PIPE >= 0 and i - _PIPE < n_groups:
                emit_matmul(i - _PIPE)

        # pinned SP endgame after batch A: deferred mid groups in order,
        # then batch B (waits the final bias-add) as the very last transfer
        for row0, og in deferred:
            dst = out[bass.ds(row0 * 128, _GROUP * 128), :].rearrange(
                "(t p) a -> p t a", p=128
            )
            nc.sync.dma_start(dst, og[:])
        dstB = out[bass.ds(first_b * 128, TB * 128), :].rearrange(
            "(t p) a -> p t a", p=128
        )
        nc.sync.dma_start(dstB, obb_t[:])

    nc.finalize()
    return nc


_NC_CACHE = None
LAST_RESULTS = None


def _get_nc():
    global _NC_CACHE
    if _NC_CACHE is None:
        _NC_CACHE = _build_nc()
    return _NC_CACHE


def _fold_weights(geodesic_weights: np.ndarray, W: np.ndarray) -> np.ndarray:
    """W' = W @ blockdiag(L(tanh(g))^T per 4-group), in float64."""
    q = np.tanh(geodesic_weights.astype(np.float64))[0]  # [N, 4]
    w_, i_, j_, k_ = q[:, 0], q[:, 1], q[:, 2], q[:, 3]
    n = q.shape[0]
    M = np.empty((n, 4, 4), dtype=np.float64)  # y_r = sum_s M[n, r, s] x_s
    M[:, 0] = np.stack([w_, -i_, -j_, -k_], axis=-1)
    M[:, 1] = np.stack([i_, w_, -k_, j_], axis=-1)
    M[:, 2] = np.stack([j_, k_, w_, -i_], axis=-1)
    M[:, 3] = np.stack([k_, -j_, i_, w_], axis=-1)
    W4 = W.astype(np.float64).reshape(A, n, 4)  # [a, n, r]
    Wp = np.einsum("anr,nrs->ans", W4, M).reshape(A, D)
    return Wp.astype(np.float32)  # [a, d]


def kernel(x, geodesic_weights, W, b, **_unused):
    x = np.ascontiguousarray(np.asarray(x, dtype=np.float32))
    Wp = _fold_weights(np.asarray(geodesic_weights), np.asarray(W))
    # device layout: w_dev[p, k*A + a] = Wp[a, 128k + p]
    w_dev = np.ascontiguousarray(
        Wp.T.reshape(KC, 128, A).transpose(1, 0, 2).reshape(128, KC * A)
    ).astype(np.float16)
    bias_dev = np.ascontiguousarray(np.asarray(b, dtype=np.float32)[None, :])

    nc = _get_nc()
    shards = np.split(x, N_CORES, axis=0)
    in_maps = [{"x": s, "w": w_dev, "bias": bias_dev} for s in shards]
    res = run_bass_kernel_spmd(
        nc,
        in_maps,
        core_ids=list(range(N_CORES)),
        trace=bool(int(os.environ.get("KERNEL_TRACE", "0"))),
    )
    global LAST_RESULTS
    LAST_RESULTS = res
    out = np.concatenate([r["out"] for r in res.results], axis=0)
    return out
